# revision 1
# baseline (speedup 1.0000x reference)
"""DifferentialMaxtree forward on 8 Trainium2 NeuronCores (Bass).

Math: node_vals[v] = sum of term[u] over v's ancestor chain (incl. v), with
term = maxtree_diff * sigmoid(feat(attributes) @ w + b); out = node_vals[pixel_node].

Implementation — Euler-tour formulation: ancestor-chain sums of ALL nodes are
inclusive prefix sums of a signed Euler sequence (+term at DFS entry, -term at
DFS exit). The tour itself (pure integer structure of maxtree_parent) is
computed on the host as sharding/layout prep; float math (features, sigmoid,
term and its negation, the 4M-slot prefix scan, the 16M-pixel table lookup)
runs on device; host glue between the two device kernels is pure index
gathers / reshapes.

Device kernel 1 (sharded over nodes): elementwise feature pipeline ->
term and -term per node (node order halves traffic vs the euler-order
baseline).
Device kernel 2 (sharded over pixels): reduce-then-scan over the full
sequence — pass 1 streams tiles for per-partition row sums, PE triangular
matmul turns them into exclusive row offsets, pass 2 rescans with the
offset folded into the scan's `initial` (carry chained across tiles, no
separate offset-add pass) and writes each P tile to DRAM as it is
scanned -> out = P[enter(pixel_node)] via per-element SWDGE indirect-DMA
gather (J=8192 per call; the out row must stay under the 64KB SDMA
descriptor limit), idx load + output flushes on the scalar-engine HWDGE
ring so they never stall the sync ring.

Measured (8 cores, axon): term ~<100us; gather kernel ~12.8ms/core —
dominated by the indirect DMA's ~6.3ns/descriptor serial cost (2M
per-element descriptors per core), which binds both this and the
baseline implementation; scan+streaming phases are ~100us.
"""

import numpy as np

N_NODES = 2 ** 21
H = W = 4096
NCORES = 8
E = 2 * N_NODES               # euler slots (4M)
NPC = N_NODES // NCORES       # nodes per core (256K)
NPP = NPC // 128              # per-partition nodes in kernel 1 (2048)
F1 = 1024                     # kernel-1 tile free size
NT1 = NPP // F1               # kernel-1 tiles (2)

ES2 = E // 128                # kernel-2 per-partition euler slots (32768)
FS = 4096                     # kernel-2 scan tile free size
NT2 = ES2 // FS               # kernel-2 scan tiles (8)

PIX = H * W // NCORES         # pixels per core (2M)
J = 8192                      # lookups per indirect gather call (out row
                              # stays under the 64KB SDMA descriptor limit)
NCALLS = PIX // J             # 256 (out_sb row c%128 per call)
GCALLS = 64                   # gather calls per flush group
NGRP = NCALLS // GCALLS       # 4

_CACHE = {}


# ---------------------------------------------------------------------------
# Host: Euler tour structure (integer work on maxtree_parent only)
# ---------------------------------------------------------------------------

def _euler_structure(par):
    n = par.shape[0]
    parc = par.astype(np.int64).copy()
    parc[0] = 0

    depth = (np.arange(n) != 0).astype(np.int64)
    cur = parc.copy()
    alive = cur != 0
    guard = 0
    while alive.any():
        depth[alive] += 1
        cur = parc[cur]
        alive = cur != 0
        guard += 1
        if guard > 100000:
            raise RuntimeError("depth loop did not converge")
    maxd = int(depth.max())

    order = np.argsort(depth, kind="stable")
    bounds = np.searchsorted(depth[order], np.arange(maxd + 2))

    size = np.ones(n, np.int64)
    for lev in range(maxd, 0, -1):
        nodes = order[bounds[lev]:bounds[lev + 1]]
        np.add.at(size, parc[nodes], size[nodes])

    # children ordered by (parent, id); exclusive prefix of sibling sizes
    o = np.argsort(par[1:], kind="stable")
    ch = np.arange(1, n, dtype=np.int64)[o]
    chp = par[1:].astype(np.int64)[o]
    csz = size[ch]
    excl = np.cumsum(csz) - csz
    grp_first = np.r_[True, chp[1:] != chp[:-1]]
    first_idx = np.maximum.accumulate(
        np.where(grp_first, np.arange(ch.shape[0]), 0))
    presib = np.zeros(n, np.int64)
    presib[ch] = excl - excl[first_idx]

    pre = np.zeros(n, np.int64)
    for lev in range(1, maxd + 1):
        nodes = order[bounds[lev]:bounds[lev + 1]]
        pre[nodes] = pre[parc[nodes]] + 1 + presib[nodes]

    enter = 2 * pre - depth
    leave = enter + 2 * size - 1
    src2 = np.empty(2 * n, np.int64)   # index into [term, -term] concat
    ar = np.arange(n)
    src2[enter] = ar
    src2[leave] = ar + n
    return src2, enter


# ---------------------------------------------------------------------------
# Device kernel 1: term / -term per node (elementwise pipeline)
# ---------------------------------------------------------------------------

def _build_term_kernel(reps=1):
    from concourse import mybir, bacc
    import concourse.tile as tile

    dt = mybir.dt.float32
    AF = mybir.ActivationFunctionType
    OP = mybir.AluOpType

    nc = bacc.Bacc("TRN2", target_bir_lowering=False, debug=False)
    att = [nc.dram_tensor(f"att{c}", [128, NPP], dt, kind="ExternalInput")
           for c in range(15)]
    dff = nc.dram_tensor("dff", [128, NPP], dt, kind="ExternalInput")
    wvec = nc.dram_tensor("wvec", [128, 22], dt, kind="ExternalInput")
    pos = nc.dram_tensor("pos", [128, NPP], dt, kind="ExternalOutput")
    neg = nc.dram_tensor("neg", [128, NPP], dt, kind="ExternalOutput")

    with tile.TileContext(nc) as tc:
        with tc.tile_pool(name="const", bufs=1) as cpool, \
             tc.tile_pool(name="work", bufs=2) as wpool, \
             tc.tile_pool(name="io", bufs=2) as iopool:
            wt = cpool.tile([128, 22], dt)
            nc.sync.dma_start(wt[:], wvec[:])
            for it in range(NT1 * reps):
                t = it % NT1
                sl = slice(t * F1, (t + 1) * F1)
                a = [iopool.tile([128, F1], dt, tag=f"a{c}", name=f"a{c}_{it}")
                     for c in range(15)]
                for c in range(15):
                    nc.sync.dma_start(a[c][:], att[c][:, sl])
                dff_t = iopool.tile([128, F1], dt, tag="dff")
                nc.sync.dma_start(dff_t[:], dff[:, sl])

                acc = wpool.tile([128, F1], dt, tag="acc")
                tmp = wpool.tile([128, F1], dt, tag="tmp")
                tmp2 = wpool.tile([128, F1], dt, tag="tmp2")

                # linear: acc = b + sum_c w_c * feat_c
                nc.vector.tensor_scalar(
                    out=acc[:], in0=a[0][:],
                    scalar1=wt[:, 0:1], scalar2=wt[:, 17:18],
                    op0=OP.mult, op1=OP.add)
                for c in range(1, 5):
                    nc.vector.scalar_tensor_tensor(
                        out=acc[:], in0=a[c][:], scalar=wt[:, c:c + 1],
                        in1=acc[:], op0=OP.mult, op1=OP.add)
                # features 5..13 = log(att[6..14] + eps)
                for c in range(5, 14):
                    nc.scalar.activation(out=tmp[:], in_=a[c + 1][:],
                                         func=AF.Ln, bias=wt[:, 18:19], scale=1.0)
                    nc.vector.scalar_tensor_tensor(
                        out=acc[:], in0=tmp[:], scalar=wt[:, c:c + 1],
                        in1=acc[:], op0=OP.mult, op1=OP.add)
                # feature 14: lshape = sqrt(a7) / (sqrt(a6) + eps)
                nc.scalar.activation(out=tmp[:], in_=a[7][:], func=AF.Sqrt)
                nc.scalar.activation(out=tmp2[:], in_=a[6][:], func=AF.Sqrt)
                nc.vector.tensor_scalar_add(out=tmp2[:], in0=tmp2[:],
                                            scalar1=wt[:, 18:19])
                nc.vector.reciprocal(out=tmp2[:], in_=tmp2[:])
                nc.vector.tensor_tensor(out=tmp[:], in0=tmp[:], in1=tmp2[:],
                                        op=OP.mult)
                nc.vector.scalar_tensor_tensor(
                    out=acc[:], in0=tmp[:], scalar=wt[:, 14:15], in1=acc[:],
                    op0=OP.mult, op1=OP.add)
                # feature 15/16: cos/sin of angle (col 5), with range reduction
                # cos(x) = sin(y), y = x + pi/2; reduce y to (-pi, pi]
                nc.vector.tensor_scalar(
                    out=tmp[:], in0=a[5][:], scalar1=wt[:, 19:20],
                    scalar2=None, op0=OP.add)             # y = x + pi/2
                nc.vector.tensor_scalar(
                    out=tmp2[:], in0=tmp[:], scalar1=wt[:, 20:21],
                    scalar2=None, op0=OP.is_gt)           # m = y > pi
                nc.vector.scalar_tensor_tensor(
                    out=tmp[:], in0=tmp2[:], scalar=wt[:, 21:22], in1=tmp[:],
                    op0=OP.mult, op1=OP.add)              # y += m * (-2pi)
                nc.scalar.activation(out=tmp[:], in_=tmp[:], func=AF.Sin)
                nc.vector.scalar_tensor_tensor(
                    out=acc[:], in0=tmp[:], scalar=wt[:, 15:16], in1=acc[:],
                    op0=OP.mult, op1=OP.add)
                # sin(x), x in [0, 2pi): reduce to (-pi, pi]
                nc.vector.tensor_scalar(
                    out=tmp2[:], in0=a[5][:], scalar1=wt[:, 20:21],
                    scalar2=None, op0=OP.is_gt)
                nc.vector.scalar_tensor_tensor(
                    out=tmp[:], in0=tmp2[:], scalar=wt[:, 21:22], in1=a[5][:],
                    op0=OP.mult, op1=OP.add)
                nc.scalar.activation(out=tmp[:], in_=tmp[:], func=AF.Sin)
                nc.vector.scalar_tensor_tensor(
                    out=acc[:], in0=tmp[:], scalar=wt[:, 16:17], in1=acc[:],
                    op0=OP.mult, op1=OP.add)
                # sigmoid, then term = cc * diff; also emit -term
                nc.scalar.activation(out=acc[:], in_=acc[:], func=AF.Sigmoid)
                outp = wpool.tile([128, F1], dt, tag="outp")
                outn = wpool.tile([128, F1], dt, tag="outn")
                nc.vector.tensor_tensor(out=outp[:], in0=acc[:], in1=dff_t[:],
                                        op=OP.mult)
                nc.vector.tensor_scalar(out=outn[:], in0=outp[:],
                                        scalar1=-1.0, scalar2=None,
                                        op0=OP.mult)
                nc.sync.dma_start(pos[:, sl], outp[:])
                nc.sync.dma_start(neg[:, sl], outn[:])
    nc.compile()
    return nc


# ---------------------------------------------------------------------------
# Device kernel 2: SBUF-resident prefix scan of full sequence + pixel gather
# ---------------------------------------------------------------------------

def _build_gather_kernel(reps=1):
    from concourse import bass, mybir, bacc

    dt = mybir.dt.float32
    OP = mybir.AluOpType

    nc = bacc.Bacc("TRN2", target_bir_lowering=False, debug=False)
    seq = nc.dram_tensor("seq", [128, ES2], dt, kind="ExternalInput")
    pix = nc.dram_tensor("pix", [128, PIX // 128], mybir.dt.int32,
                         kind="ExternalInput")
    ptab = nc.dram_tensor("ptab", [E, 1], dt, kind="Internal")
    out = nc.dram_tensor("out", [NCALLS, J], dt, kind="ExternalOutput")
    ptab2d = ptab[:].rearrange("(p f) one -> p (f one)", p=128)

    from contextlib import ExitStack
    with (
        ExitStack() as ctx,
        nc.Block() as block,
        nc.sbuf_tensor("stile", [128, 2 * FS], dt) as stile,
        nc.sbuf_tensor("carry", [128, 1], dt) as carry,
        nc.sbuf_tensor("rsum", [128, 1], dt) as rsum,
        nc.sbuf_tensor("rtmp", [128, 1], dt) as rtmp,
        nc.sbuf_tensor("tri", [128, 128], dt) as tri,
        nc.sbuf_tensor("idx_sb", [128, PIX // 128], mybir.dt.int32) as idx_sb,
        nc.sbuf_tensor("out_sb", [128, J], dt) as out_sb,
        nc.psum_tensor("pacc", [128, 1], dt) as pacc,
    ):
        def sem(name):
            return ctx.enter_context(nc.semaphore(name))  # noqa: ANT232
        s_a = (sem("s_a0"), sem("s_a1"))   # pass-1 tile loaded (even/odd)
        s_rd = sem("s_rd")     # pass-1 tile consumed (rsum updated)
        s_b = (sem("s_b0"), sem("s_b1"))   # pass-2 tile loaded (even/odd)
        s_v = sem("s_v")       # DVE completion chain (RAW barriers)
        s_tri = sem("s_tri")
        s_mm = sem("s_mm")
        s_po = (sem("s_po0"), sem("s_po1"))  # ptab tile written (even/odd)
        s_idx = sem("s_idx")   # idx loaded
        s_g = sem("s_g")       # gathers done
        s_f = sem("s_f")       # flushes done

        # per-rep semaphore deltas (reps>1 builds are timing-only)
        PA = 16 * (NT2 // 2)    # s_a / s_b / s_po parity streams
        RD = NT2                # s_rd
        V = 3 * NT2 + 1         # s_v: reduces + pacc copy + scan/carry pairs
        G = 16 * NCALLS         # s_g
        F = 16 * NGRP           # s_f

        # s_v schedule within a rep: reduces -> 1..NT2; pacc copy -> NT2+1;
        # per pass-2 tile t: scan -> NT2+2+2t, carry copy -> NT2+3+2t.
        def SV_SCAN(r, t):
            return r * V + NT2 + 2 + 2 * t

        @block.sync
        def _(sync):
            for r in range(reps):
                # pass 1: stream tiles for row-sum accumulation
                for t in range(NT2):
                    if r > 0:
                        sync.wait_ge(s_po[t % 2], r * PA)
                    if t >= 2:
                        sync.wait_ge(s_rd, r * RD + t - 1)
                    sync.dma_start(
                        stile[:, (t % 2) * FS:(t % 2 + 1) * FS],
                        seq[:, t * FS:(t + 1) * FS],
                    ).then_inc(s_a[t % 2], 16)
                # pass 2: reload tiles; write ptab as each tile is scanned
                for t in range(NT2):
                    if t < 2:
                        sync.wait_ge(s_rd, (r + 1) * RD)  # pass-1 consumed
                    else:
                        sync.wait_ge(s_po[t % 2],
                                     r * PA + 16 * ((t - 2) // 2 + 1))
                    sync.dma_start(
                        stile[:, (t % 2) * FS:(t % 2 + 1) * FS],
                        seq[:, t * FS:(t + 1) * FS],
                    ).then_inc(s_b[t % 2], 16)
                    sync.wait_ge(s_v, SV_SCAN(r, t))
                    sync.dma_start(
                        ptab2d[:, t * FS:(t + 1) * FS],
                        stile[:, (t % 2) * FS:(t % 2 + 1) * FS],
                    ).then_inc(s_po[t % 2], 16)
                sync.wait_ge(s_f, (r + 1) * F)

        @block.scalar
        def _(scalar):
            for r in range(reps):
                # idx load on the ACT HWDGE ring (off the sync ring)
                if r > 0:
                    scalar.wait_ge(s_g, r * G)
                scalar.dma_start(idx_sb[:], pix[:]).then_inc(s_idx, 16)
                # flushes of gathered row-halves (HWDGE; own queue)
                for g in range(NGRP):
                    scalar.wait_ge(s_g, r * G + 16 * GCALLS * (g + 1))
                    scalar.dma_start(
                        out[g * GCALLS:(g + 1) * GCALLS, :],
                        out_sb[(g % 2) * GCALLS:(g % 2 + 1) * GCALLS, :],
                    ).then_inc(s_f, 16)

        @block.vector
        def _(vector):
            for r in range(reps):
                if r > 0:
                    vector.wait_ge(s_mm, r)   # WAR: prev matmul read rsum
                vector.memset(rsum[:], 0.0)
                for t in range(NT2):
                    vector.wait_ge(s_a[t % 2], r * PA + 16 * (t // 2 + 1))
                    if t > 0:
                        vector.wait_ge(s_rd, r * RD + t)  # WAR on rtmp
                    buf = stile[:, (t % 2) * FS:(t % 2 + 1) * FS]
                    vector.tensor_reduce(out=rtmp[:], in_=buf,
                                         axis=mybir.AxisListType.X,
                                         op=OP.add).then_inc(s_v, 1)
                    vector.wait_ge(s_v, r * V + t + 1)    # RAW on rtmp
                    vector.tensor_tensor(out=rsum[:], in0=rsum[:],
                                         in1=rtmp[:],
                                         op=OP.add).then_inc(s_rd, 1)
                # cross-partition exclusive row offsets -> initial carry
                vector.wait_ge(s_mm, r + 1)
                vector.tensor_copy(out=carry[:],
                                   in_=pacc[:, 0:1]).then_inc(s_v, 1)
                # pass 2: chained scans; carry holds prev tile's last value
                for t in range(NT2):
                    vector.wait_ge(s_b[t % 2], r * PA + 16 * (t // 2 + 1))
                    vector.wait_ge(s_v, SV_SCAN(r, t) - 1)  # carry final
                    buf = stile[:, (t % 2) * FS:(t % 2 + 1) * FS]
                    vector.tensor_tensor_scan(
                        out=buf, data0=buf, data1=buf,
                        initial=carry[:], op0=OP.add, op1=OP.bypass,
                    ).then_inc(s_v, 1)
                    vector.wait_ge(s_v, SV_SCAN(r, t))      # scan committed
                    vector.tensor_copy(
                        out=carry[:],
                        in_=stile[:, (t % 2) * FS + FS - 1:(t % 2) * FS + FS],
                    ).then_inc(s_v, 1)

        @block.tensor
        def _(pe):
            pe.wait_ge(s_tri, 2)
            for r in range(reps):
                if r > 0:
                    pe.wait_ge(s_v, r * V)    # WAR: prev pacc copy done
                pe.wait_ge(s_rd, (r + 1) * RD)
                pe.matmul(out=pacc[:, 0:1], lhsT=tri[:], rhs=rsum[:],
                          start=True, stop=True).then_inc(s_mm, 1)

        @block.gpsimd
        def _(gpsimd):
            # strictly-upper-triangular ones; explicit barrier between the
            # memset and the select (make_upper_triangular emits none)
            gpsimd.memset(tri[:], 0.0).then_inc(s_tri, 1)
            gpsimd.wait_ge(s_tri, 1)
            gpsimd.affine_select(
                out=tri[:], in_=tri[:],
                compare_op=mybir.AluOpType.is_ge,
                fill=1.0, base=0,
                pattern=[[-1, 128]], channel_multiplier=1,
            ).then_inc(s_tri, 1)
            for r in range(reps):
                gpsimd.wait_ge(s_po[0], (r + 1) * PA)
                gpsimd.wait_ge(s_po[1], (r + 1) * PA)
                gpsimd.wait_ge(s_idx, 16 * (r + 1))
                for c in range(NCALLS):
                    g = c // GCALLS
                    if g >= 2:
                        gpsimd.wait_ge(s_f, r * F + 16 * (g - 1))
                    row = c % 128
                    gpsimd.indirect_dma_start(
                        out=out_sb[row:row + 1, :].unsqueeze(-1),
                        out_offset=None,
                        in_=ptab[:],
                        in_offset=bass.IndirectOffsetOnAxis(
                            ap=idx_sb[:, c * (J // 128):(c + 1) * (J // 128)],
                            axis=0),
                    ).then_inc(s_g, 16)
            gpsimd.wait_ge(s_g, reps * G)

    nc.compile()
    return nc


# ---------------------------------------------------------------------------
# Entry point
# ---------------------------------------------------------------------------

def _prep_inputs(inputs):
    diff = np.asarray(inputs["maxtree_diff"], np.float32)
    attributes = np.asarray(inputs["attributes"], np.float32)
    weight = np.asarray(inputs["weight"], np.float32)
    bias = np.asarray(inputs["bias"], np.float32)
    parent = np.asarray(inputs["maxtree_parent"], np.int32)
    pixel_node = np.asarray(inputs["pixel_node"], np.int32)

    src2, enter = _euler_structure(parent)

    wv = np.zeros((128, 22), np.float32)
    wv[:, :17] = weight[:, 0][None, :]
    wv[:, 17] = bias[0]
    wv[:, 18] = 1e-10
    wv[:, 19] = np.float32(np.pi / 2)
    wv[:, 20] = np.float32(np.pi)
    wv[:, 21] = np.float32(-2 * np.pi)

    attT = np.ascontiguousarray(attributes.T)          # (15, N)
    in1 = []
    for c in range(NCORES):
        sl = slice(c * NPC, (c + 1) * NPC)
        m = {f"att{k}": np.ascontiguousarray(
                attT[k, sl].reshape(128, NPP)) for k in range(15)}
        m["dff"] = np.ascontiguousarray(diff[sl].reshape(128, NPP))
        m["wvec"] = wv
        in1.append(m)

    eidx_img = enter[pixel_node].astype(np.int32)      # (H, W)
    pix_maps = []
    for c in range(NCORES):
        flat = eidx_img[c * (H // NCORES):(c + 1) * (H // NCORES), :]
        # call c gathers offset ap idx[:, c*128:(c+1)*128] into out row c as
        # a partition-fastest stream: out[c][k] = ptab[idx[k%128, c*128+k//128]]
        arr = flat.reshape(NCALLS, J // 128, 128).transpose(2, 0, 1)
        pix_maps.append(np.ascontiguousarray(arr.reshape(128, PIX // 128)))
    return in1, pix_maps, src2


def _seq_from_term(res1, src2):
    both = np.concatenate(
        [np.concatenate([res1.results[c]["pos"].reshape(-1)
                         for c in range(NCORES)]),
         np.concatenate([res1.results[c]["neg"].reshape(-1)
                         for c in range(NCORES)])])
    seq_full = both[src2]                              # pure index gather
    return np.ascontiguousarray(seq_full.reshape(128, ES2))


def _img_from_out(res2):
    rows = [res2.results[c]["out"].reshape(H // NCORES, W)
            for c in range(NCORES)]
    return np.concatenate(rows, axis=0)


def kernel(**inputs):
    from concourse.bass_utils import run_bass_kernel_spmd

    in1, pix_maps, src2 = _prep_inputs(inputs)

    if "term" not in _CACHE:
        _CACHE["term"] = _build_term_kernel()
    if "gather" not in _CACHE:
        _CACHE["gather"] = _build_gather_kernel()

    res1 = run_bass_kernel_spmd(_CACHE["term"], in1,
                                core_ids=list(range(NCORES)))
    seq2d = _seq_from_term(res1, src2)
    in2 = [{"seq": seq2d, "pix": pix_maps[c]} for c in range(NCORES)]

    res2 = run_bass_kernel_spmd(_CACHE["gather"], in2,
                                core_ids=list(range(NCORES)))
    _CACHE["in1"], _CACHE["in2"] = in1, in2
    return _img_from_out(res2)


def timed_run(inputs, trace=False):
    """Predicted + in-kernel-repetition HW slope time for both kernels."""
    import bench

    if "in1" not in _CACHE:
        kernel(**inputs)
    p1 = bench.predict_ns(_CACHE["term"])
    p2 = bench.predict_ns(_CACHE["gather"])
    print(f"predicted: term {p1:.0f} ns, gather {p2:.0f} ns, "
          f"total {p1 + p2:.0f} ns")

    def slope(build, in_maps, r_lo, r_hi, label, fallback):
        try:
            t = {}
            for r in (r_lo, r_hi):
                nc = build(reps=r)
                t[r] = bench.hw_bench_ns(nc, in_maps, NCORES, iters=8,
                                         label=f"{label} R={r}")
            return (t[r_hi] - t[r_lo]) / (r_hi - r_lo)
        except Exception as e:
            print(f"{label} slope bench failed ({type(e).__name__}); "
                  f"using cost-model fallback")
            return fallback

    t1 = slope(_build_term_kernel, _CACHE["in1"], 2, 10, "term", p1)
    t2 = slope(_build_gather_kernel, _CACHE["in2"], 2, 10, "gather", p2)
    print(f"hw-slope: term {t1:.0f} ns, gather {t2:.0f} ns")
    return int(t1 + t2)

